# revision 2
# baseline (speedup 1.0000x reference)
"""Dense-MoE (top-2 of 8 experts) TRN2 kernel v5: expert-parallel, bf16 matmuls.

Host side: softmax + top-2 routing, per-expert token gather (padded to
cap_pad = nch*ck, chunk-major layout), weight re-layout + bf16 conversion.
Device side (per core = one expert), all matmul operands bf16, PSUM f32:
    phase A:  h[f, c] = silu(gw @ x) * (uw @ x)      [f-major, bf16 in SBUF]
    phase B:  outT[d, c] = sum_f dwT[f, d] * h[f, c]  [tokens on the free dim]
The routing weight and the scatter-add back to [T, D] happen on the host
(out[t] += w_t * outT[:, c].T).

vs v4 (193.2us): attacks the ramp and tail, leaving the near-roofline
steady state (97.8% of 2.38 GHz issue) alone.
  - x is laid out CHUNK-major ([P, nch, D/P, ck]) and DMAed per chunk, so
    the first gate chain needs only gw0 (0.25 MB) + x-chunk0 (0.7 MB)
    instead of gw0/uw0 + all of x (2.9 MB): first matmul ~3us earlier.
    Phase A runs ci-outer / dt-chain inner (gate chain then up chain), so
    uw0 and later chunks get progressively wider DMA windows.
  - PE p-state prewarm: the PE starts at the MID p-state (2x slower,
    0.833 ns/col, the measured 427ns/512col early cadence) and only
    reaches full speed after ~3us of continuous work. ~7 dummy 512-col
    matmuls on a memset tile run during the initial DMA window so real
    matmuls start at full speed.
  - Tail: the last d-tile of phase B splits its final chunk in two, so the
    kernel-ending PSUM drain + out DMA is half-sized; its accumulation
    groups close at staggered intervals (sequential chunk order) as in v4.
Fixed costs kept in mind: ~8us of NEFF semaphore-teardown after the last
DMA is included in the measured window and is not kernel-addressable;
DMA descriptor issue costs ~0.6us per descriptor on the sync/scalar
hardware-DGE queues (so x streams as 4 descriptors, weights as 1 each).
fp8 was investigated and is numerically out of reach: DoubleRow fp8 is
exactly 2x bf16 on HW, but pure-fp8 absmax error is 7e-2 vs the 2e-2
budget, and hi/lo-compensated fp8 (3 products) is 1.5x bf16 time.
"""
import sys

sys.path.insert(0, "/opt/trn_rl_repo")

import ml_dtypes
import numpy as np

import concourse.bass as bass
from concourse import bacc
import concourse.mybir as mybir
import concourse.tile as tile
from concourse.bass_utils import run_bass_kernel_spmd
from concourse.bass import ds

T, D, F, E, TOPK = 4096, 1024, 2048, 8, 2
P = 128
N_CORES = 8

F32 = mybir.dt.float32
BF16 = mybir.dt.bfloat16
BF = ml_dtypes.bfloat16

N_WARM = 7  # dummy 512-col matmuls to ramp the PE p-state during DMA wait


def _build(nch, ck):
    """nch chunks of ck columns each; cap_pad = nch*ck tokens per core."""
    cap = nch * ck
    assert nch <= 3, f"{nch} chunks > 3 (PSUM banks)"
    assert ck <= 512

    nc = bacc.Bacc(None, target_bir_lowering=False)
    x_d = nc.declare_dram_parameter("x", [P, nch, D // P, ck], BF16, isOutput=False)
    gw_d = nc.declare_dram_parameter("gw", [P, F // P, D // P, P], BF16, isOutput=False)
    uw_d = nc.declare_dram_parameter("uw", [P, F // P, D // P, P], BF16, isOutput=False)
    dw_d = nc.declare_dram_parameter("dw", [P, F // P, D], BF16, isOutput=False)
    out_d = nc.declare_dram_parameter("out", [P, D // P, cap], BF16, isOutput=True)

    with tile.TileContext(nc) as tc:
        with (
            tc.tile_pool(name="deep", bufs=1) as deep,
            tc.tile_pool(name="wts", bufs=3) as wts,
            tc.tile_pool(name="stage", bufs=2) as stage,
            tc.tile_pool(name="ps", bufs=1, space="PSUM") as ps,
        ):
            # --- PE p-state prewarm: memset a scratch tile (vector engine is
            # idle at start), then dummy matmuls into two scratch PSUM banks.
            warm_sb = deep.tile([P, 640], BF16, tag="warm")
            nc.vector.memset(warm_sb[:], 0)
            warm_ps = [ps.tile([P, 512], F32, tag=f"w{i}", name=f"w{i}") for i in range(2)]
            for i in range(N_WARM):
                nc.tensor.matmul(
                    warm_ps[i % 2][:], warm_sb[:, :P], warm_sb[:, ds(P, 512)],
                    start=True, stop=True,
                )

            wt_tiles = {}

            def load_ft(ft):
                gw_t = wts.tile([P, D // P, P], BF16, tag="gw", name="gw_t")
                nc.sync.dma_start(gw_t[:], gw_d[:, ft])
                uw_t = wts.tile([P, D // P, P], BF16, tag="uw", name="uw_t")
                nc.sync.dma_start(uw_t[:], uw_d[:, ft])
                wt_tiles[ft] = (gw_t, uw_t)

            # Startup DMAs in PE consumption order. Weights on the sync queue,
            # x on the scalar queue (the two hardware-DGE queues issue in
            # parallel; ~0.6us per descriptor). x chunk0 is split in two so
            # the first gate chain's early d-slices can start sooner.
            load_ft(0)
            x_t = deep.tile([P, nch, D // P, ck], BF16, tag="x")
            half = (D // P) // 2
            nc.scalar.dma_start(x_t[:, 0, ds(0, half)], x_d[:, 0, ds(0, half)])
            nc.scalar.dma_start(x_t[:, 0, ds(half, half)], x_d[:, 0, ds(half, half)])
            for ci in range(1, nch):
                nc.scalar.dma_start(x_t[:, ci], x_d[:, ci])
            load_ft(1)

            h_t = deep.tile([P, F // P, cap], BF16, tag="h")
            dw_t = deep.tile([P, F // P, D], BF16, tag="dw")

            # Phase A: h[fp, ft, c] = silu(gw@x) * (uw@x); chunk-outer,
            # gate chain then up chain (widens the uw/x DMA windows).
            for ft in range(F // P):
                if ft == 8:
                    for fo in range(0, F // P, 4):
                        nc.sync.dma_start(dw_t[:, ds(fo, 4)], dw_d[:, ds(fo, 4)])
                if ft + 1 < F // P and (ft + 1) not in wt_tiles:
                    load_ft(ft + 1)
                gw_t, uw_t = wt_tiles.pop(ft)
                for ci in range(nch):
                    pg = ps.tile([P, 512], F32, tag=f"pg{ci}", name=f"pg{ci}")
                    pu = ps.tile([P, 512], F32, tag=f"pu{ci}", name=f"pu{ci}")
                    for dt_ in range(D // P):
                        nc.tensor.matmul(
                            pg[:, :ck], gw_t[:, dt_], x_t[:, ci, dt_],
                            start=(dt_ == 0), stop=(dt_ == D // P - 1),
                        )
                    for dt_ in range(D // P):
                        nc.tensor.matmul(
                            pu[:, :ck], uw_t[:, dt_], x_t[:, ci, dt_],
                            start=(dt_ == 0), stop=(dt_ == D // P - 1),
                        )
                    sg = stage.tile([P, 512], BF16, tag=f"sg{ci}", name=f"sg{ci}")
                    nc.scalar.activation(sg[:, :ck], pg[:, :ck],
                                         mybir.ActivationFunctionType.Silu)
                    nc.vector.tensor_tensor(
                        h_t[:, ft, ds(ci * ck, ck)], sg[:, :ck], pu[:, :ck],
                        mybir.AluOpType.mult,
                    )

            # Phase B: outT[dp, dt, c] = sum_f dwT[f, d] * h[f, c].
            # Output banks alternate between the pg* and pu* tag sets so the
            # next d-tile's accumulation doesn't wait on this one's drain.
            for dt_ in range(D // P):
                grp, other = ("pg", "pu") if dt_ % 2 == 0 else ("pu", "pg")
                last = dt_ == D // P - 1
                if last:
                    # Sequential chunks with the final chunk split in two:
                    # each accumulation group closes at a staggered interval
                    # so its drain copy + DMA overlap the remaining chunks'
                    # matmuls, and the kernel-ending drain is half-sized.
                    ck2 = (ck // 2) + ((ck // 2) & 1)
                    chunks = [(ci * ck, ck) for ci in range(nch - 1)]
                    chunks += [((nch - 1) * ck, ck2), ((nch - 1) * ck + ck2, ck - ck2)]
                    tags = [f"{grp}{ci}" for ci in range(nch)] + [f"{other}0"]
                    osb = stage.tile([P, cap], BF16, tag="osb", name="osb")
                    for ci, (c0, cs) in enumerate(chunks):
                        po = ps.tile([P, 512], F32, tag=tags[ci], name=f"po{ci}")
                        for fo in range(F // P):
                            nc.tensor.matmul(
                                po[:, :cs], dw_t[:, fo, ds(dt_ * P, P)],
                                h_t[:, fo, ds(c0, cs)],
                                start=(fo == 0), stop=(fo == F // P - 1),
                            )
                        if ci % 2 == 0:
                            nc.scalar.activation(osb[:, ds(c0, cs)], po[:, :cs],
                                                 mybir.ActivationFunctionType.Copy)
                        else:
                            nc.vector.tensor_scalar_mul(osb[:, ds(c0, cs)], po[:, :cs], 1.0)
                        # Per-chunk DMA: subtile deps release each slice as
                        # soon as its copy lands.
                        nc.sync.dma_start(out_d[:, dt_, ds(c0, cs)], osb[:, ds(c0, cs)])
                else:
                    pos = [ps.tile([P, 512], F32, tag=f"{grp}{ci}", name=f"po{ci}")
                           for ci in range(nch)]
                    for fo in range(F // P):
                        for ci in range(nch):
                            nc.tensor.matmul(
                                pos[ci][:, :ck], dw_t[:, fo, ds(dt_ * P, P)],
                                h_t[:, fo, ds(ci * ck, ck)],
                                start=(fo == 0), stop=(fo == F // P - 1),
                            )
                    osb = stage.tile([P, cap], BF16, tag="osb", name="osb")
                    for ci in range(nch):
                        if ci % 2 == 0:
                            nc.scalar.activation(osb[:, ds(ci * ck, ck)], pos[ci][:, :ck],
                                                 mybir.ActivationFunctionType.Copy)
                        else:
                            nc.vector.tensor_scalar_mul(osb[:, ds(ci * ck, ck)], pos[ci][:, :ck], 1.0)
                    nc.sync.dma_start(out_d[:, dt_], osb[:])
    nc.finalize()
    return nc


def _route(gating_output):
    """Numpy softmax + top-2 + renormalize; returns (ids [T,K], w [T,K])."""
    g = gating_output.astype(np.float32)
    m = g.max(axis=-1, keepdims=True)
    e = np.exp(g - m)
    probs = e / e.sum(axis=-1, keepdims=True)
    ids = np.argsort(-probs, axis=-1, kind="stable")[:, :TOPK]
    w = np.take_along_axis(probs, ids, axis=-1)
    w = w / w.sum(axis=-1, keepdims=True)
    return ids, w


def kernel(x, gating_output, gate_w, up_w, down_w):
    x = np.asarray(x, dtype=np.float32)
    gating_output = np.asarray(gating_output, dtype=np.float32)
    gate_w = np.asarray(gate_w, dtype=np.float32)
    up_w = np.asarray(up_w, dtype=np.float32)
    down_w = np.asarray(down_w, dtype=np.float32)

    ids, w = _route(gating_output)

    idx_e = []
    w_e = []
    for e in range(E):
        sel = np.nonzero((ids == e).any(axis=-1))[0]
        kpos = (ids[sel] == e).argmax(axis=-1)
        idx_e.append(sel)
        w_e.append(w[sel, kpos])

    cap = max(len(i) for i in idx_e)
    nch = max(1, -(-cap // 512))  # token chunks (<=512 fp32 psum free dim)
    ck = -(-cap // nch)
    ck += ck & 1  # even chunk size
    cap_pad = nch * ck

    nc = _build(nch, ck)

    in_maps = []
    for e in range(E):
        idx = idx_e[e]
        cnt = len(idx)
        x_pad = np.zeros((cap_pad, D), dtype=np.float32)
        x_pad[:cnt] = x[idx]

        # x: [cap_pad, D] -> [128(dp), nch(ci), D/128(do), ck(c)] chunk-major
        x_dev = np.ascontiguousarray(
            x_pad.reshape(nch, ck, D // P, P).transpose(3, 0, 2, 1)).astype(BF)
        # gate/up: [F, D] -> T -> [D, F] -> [128(dp), 16(ft), 8(do), 128(fi)]
        gwT = gate_w[e].T  # [D, F]
        gw_dev = np.ascontiguousarray(
            gwT.reshape(D // P, P, F // P, P).transpose(1, 2, 0, 3)).astype(BF)
        uwT = up_w[e].T
        uw_dev = np.ascontiguousarray(
            uwT.reshape(D // P, P, F // P, P).transpose(1, 2, 0, 3)).astype(BF)
        # down: [D, F] -> T -> [F, D] -> [128(fp), 16(fo), D]
        dwT = down_w[e].T  # [F, D]
        dw_dev = np.ascontiguousarray(
            dwT.reshape(F // P, P, D).transpose(1, 0, 2)).astype(BF)

        in_maps.append({"x": x_dev, "gw": gw_dev, "uw": uw_dev, "dw": dw_dev})

    def _run():
        try:
            return run_bass_kernel_spmd(nc, in_maps, core_ids=list(range(N_CORES)))
        except Exception:
            # First execution of a fresh NEFF occasionally dies with
            # NRT_EXEC_UNIT_UNRECOVERABLE on this setup; the retry reuses
            # the cached executable and goes through.
            import time as _time

            _time.sleep(5)
            return run_bass_kernel_spmd(nc, in_maps, core_ids=list(range(N_CORES)))

    def _assemble(res):
        out = np.zeros((T, D), dtype=np.float32)
        for e in range(E):
            cnt = len(idx_e[e])
            # device out: [dp, do, c] -> [c, do*128+dp]
            o = res.results[e]["out"].astype(np.float32).transpose(2, 1, 0).reshape(cap_pad, D)
            out[idx_e[e]] += o[:cnt] * w_e[e][:, None]
        return out

    def _spot_check(out):
        # One token per (non-empty) expert, host-computed in f32. Catches the
        # rare corrupted execution (seen once: silently wrong rows on a fresh
        # NEFF) -- bf16 rounding keeps honest rows well under the threshold.
        worst = 0.0
        for e in range(E):
            if len(idx_e[e]) == 0:
                continue
            t = int(idx_e[e][0])
            acc = np.zeros(D, dtype=np.float32)
            for k in range(TOPK):
                ek = int(ids[t, k])
                g = gate_w[ek] @ x[t]
                u = up_w[ek] @ x[t]
                h = (g / (1.0 + np.exp(-g))) * u
                acc += w[t, k] * (down_w[ek] @ h)
            scale = np.abs(acc).max() + 1e-6
            worst = max(worst, np.abs(out[t] - acc).max() / scale)
        return worst

    res = _run()
    out = _assemble(res)
    if _spot_check(out) > 0.1:
        res = _run()
        out = _assemble(res)
    return out


# revision 4
# speedup vs baseline: 1.0050x; 1.0050x over previous
"""Dense-MoE (top-2 of 8 experts) TRN2 kernel v5: expert-parallel, bf16 matmuls.

Host side: softmax + top-2 routing, per-expert token gather (padded to
cap_pad = nch*ck, chunk-major layout), weight re-layout + bf16 conversion.
Device side (per core = one expert), all matmul operands bf16, PSUM f32:
    phase A:  h[f, c] = silu(gw @ x) * (uw @ x)      [f-major, bf16 in SBUF]
    phase B:  outT[d, c] = sum_f dwT[f, d] * h[f, c]  [tokens on the free dim]
The routing weight and the scatter-add back to [T, D] happen on the host
(out[t] += w_t * outT[:, c].T).

vs v4 (193.2us): attacks the ramp and tail, leaving the near-roofline
steady state (97.8% of 2.38 GHz issue) alone.
  - x is laid out CHUNK-major ([P, nch, D/P, ck]) and DMAed per chunk, so
    the first gate chain needs only gw0 (0.25 MB) + x-chunk0 (0.7 MB)
    instead of gw0/uw0 + all of x (2.9 MB): first matmul ~3us earlier.
    Phase A runs ci-outer / dt-chain inner (gate chain then up chain), so
    uw0 and later chunks get progressively wider DMA windows.
  - PE p-state prewarm: the PE starts at the MID p-state (2x slower,
    0.833 ns/col, the measured 427ns/512col early cadence) and only
    reaches full speed after ~3us of continuous work. ~7 dummy 512-col
    matmuls on a memset tile run during the initial DMA window so real
    matmuls start at full speed.
  - Tail: the last d-tile of phase B splits its final chunk in two, so the
    kernel-ending PSUM drain + out DMA is half-sized; its accumulation
    groups close at staggered intervals (sequential chunk order) as in v4.
Fixed costs kept in mind: ~8us of NEFF semaphore-teardown after the last
DMA is included in the measured window and is not kernel-addressable;
DMA descriptor issue costs ~0.6us per descriptor on the sync/scalar
hardware-DGE queues (so x streams as 4 descriptors, weights as 1 each).
fp8 was investigated and is numerically out of reach: DoubleRow fp8 is
exactly 2x bf16 on HW, but pure-fp8 absmax error is 7e-2 vs the 2e-2
budget, and hi/lo-compensated fp8 (3 products) is 1.5x bf16 time.
"""
import sys

sys.path.insert(0, "/opt/trn_rl_repo")

import ml_dtypes
import numpy as np

import concourse.bass as bass
from concourse import bacc
import concourse.mybir as mybir
import concourse.tile as tile
from concourse.bass_utils import run_bass_kernel_spmd
from concourse.bass import ds

T, D, F, E, TOPK = 4096, 1024, 2048, 8, 2
P = 128
N_CORES = 8

F32 = mybir.dt.float32
BF16 = mybir.dt.bfloat16
BF = ml_dtypes.bfloat16

N_WARM = 7  # dummy 512-col matmuls to ramp the PE p-state during DMA wait


def _build(nch, ck):
    """nch chunks of ck columns each; cap_pad = nch*ck tokens per core."""
    cap = nch * ck
    assert nch <= 3, f"{nch} chunks > 3 (PSUM banks)"
    assert ck <= 512

    nc = bacc.Bacc(None, target_bir_lowering=False)
    x_d = nc.declare_dram_parameter("x", [P, nch, D // P, ck], BF16, isOutput=False)
    gw_d = nc.declare_dram_parameter("gw", [P, F // P, D // P, P], BF16, isOutput=False)
    uw_d = nc.declare_dram_parameter("uw", [P, F // P, D // P, P], BF16, isOutput=False)
    dw_d = nc.declare_dram_parameter("dw", [P, F // P, D], BF16, isOutput=False)
    out_d = nc.declare_dram_parameter("out", [P, D // P, cap], BF16, isOutput=True)

    with tile.TileContext(nc) as tc:
        with (
            tc.tile_pool(name="deep", bufs=1) as deep,
            tc.tile_pool(name="wts", bufs=3) as wts,
            tc.tile_pool(name="stage", bufs=2) as stage,
            tc.tile_pool(name="ps", bufs=1, space="PSUM") as ps,
        ):
            # --- PE p-state prewarm: memset a scratch tile (gpsimd is the
            # least-loaded engine at start; vector's memset + sem propagation
            # cost ~2.2us in v5), then dummy matmuls into two scratch PSUM
            # banks. Real matmul data can't land before ~4us (descriptor
            # issue + HBM), so ~3us of dummies ramps the p-state for free.
            warm_sb = deep.tile([P, 640], BF16, tag="warm")
            nc.gpsimd.memset(warm_sb[:], 0)
            warm_ps = [ps.tile([P, 512], F32, tag=f"w{i}", name=f"w{i}") for i in range(2)]
            for i in range(N_WARM):
                nc.tensor.matmul(
                    warm_ps[i % 2][:], warm_sb[:, :P], warm_sb[:, ds(P, 512)],
                    start=True, stop=True,
                )

            wt_tiles = {}

            def load_ft(ft):
                gw_t = wts.tile([P, D // P, P], BF16, tag="gw", name="gw_t")
                nc.sync.dma_start(gw_t[:], gw_d[:, ft])
                uw_t = wts.tile([P, D // P, P], BF16, tag="uw", name="uw_t")
                nc.sync.dma_start(uw_t[:], uw_d[:, ft])
                wt_tiles[ft] = (gw_t, uw_t)

            # Startup DMAs in PE consumption order across BOTH hardware-DGE
            # queues (each sustains only ~200 GB/s; ~0.6us per descriptor):
            # sync gets gw0, uw0, xc1, then the ft1+ weights; scalar gets
            # xc0 (split in two so the first gate chain's early d-slices can
            # start sooner) and the remaining chunks.
            load_ft(0)
            x_t = deep.tile([P, nch, D // P, ck], BF16, tag="x")
            half = (D // P) // 2
            nc.scalar.dma_start(x_t[:, 0, ds(0, half)], x_d[:, 0, ds(0, half)])
            nc.scalar.dma_start(x_t[:, 0, ds(half, half)], x_d[:, 0, ds(half, half)])
            if nch > 1:
                nc.sync.dma_start(x_t[:, 1], x_d[:, 1])
            for ci in range(2, nch):
                nc.scalar.dma_start(x_t[:, ci], x_d[:, ci])
            load_ft(1)

            h_t = deep.tile([P, F // P, cap], BF16, tag="h")
            dw_t = deep.tile([P, F // P, D], BF16, tag="dw")

            # Phase A: h[fp, ft, c] = silu(gw@x) * (uw@x); chunk-outer,
            # gate chain then up chain (widens the uw/x DMA windows).
            for ft in range(F // P):
                if ft == 8:
                    for fo in range(0, F // P, 4):
                        nc.sync.dma_start(dw_t[:, ds(fo, 4)], dw_d[:, ds(fo, 4)])
                if ft + 1 < F // P and (ft + 1) not in wt_tiles:
                    load_ft(ft + 1)
                gw_t, uw_t = wt_tiles.pop(ft)
                for ci in range(nch):
                    pg = ps.tile([P, 512], F32, tag=f"pg{ci}", name=f"pg{ci}")
                    pu = ps.tile([P, 512], F32, tag=f"pu{ci}", name=f"pu{ci}")
                    for dt_ in range(D // P):
                        nc.tensor.matmul(
                            pg[:, :ck], gw_t[:, dt_], x_t[:, ci, dt_],
                            start=(dt_ == 0), stop=(dt_ == D // P - 1),
                        )
                    for dt_ in range(D // P):
                        nc.tensor.matmul(
                            pu[:, :ck], uw_t[:, dt_], x_t[:, ci, dt_],
                            start=(dt_ == 0), stop=(dt_ == D // P - 1),
                        )
                    sg = stage.tile([P, 512], BF16, tag=f"sg{ci}", name=f"sg{ci}")
                    nc.scalar.activation(sg[:, :ck], pg[:, :ck],
                                         mybir.ActivationFunctionType.Silu)
                    nc.vector.tensor_tensor(
                        h_t[:, ft, ds(ci * ck, ck)], sg[:, :ck], pu[:, :ck],
                        mybir.AluOpType.mult,
                    )

            # Phase B: outT[dp, dt, c] = sum_f dwT[f, d] * h[f, c].
            # Output banks alternate between the pg* and pu* tag sets so the
            # next d-tile's accumulation doesn't wait on this one's drain.
            for dt_ in range(D // P):
                grp, other = ("pg", "pu") if dt_ % 2 == 0 else ("pu", "pg")
                last = dt_ == D // P - 1
                if last:
                    # Sequential chunks with the final chunk split in two:
                    # each accumulation group closes at a staggered interval
                    # so its drain copy + DMA overlap the remaining chunks'
                    # matmuls, and the kernel-ending drain is half-sized.
                    ck2 = (ck // 2) + ((ck // 2) & 1)
                    chunks = [(ci * ck, ck) for ci in range(nch - 1)]
                    chunks += [((nch - 1) * ck, ck2), ((nch - 1) * ck + ck2, ck - ck2)]
                    tags = [f"{grp}{ci}" for ci in range(nch)] + [f"{other}0"]
                    osb = stage.tile([P, cap], BF16, tag="osb", name="osb")
                    for ci, (c0, cs) in enumerate(chunks):
                        po = ps.tile([P, 512], F32, tag=tags[ci], name=f"po{ci}")
                        for fo in range(F // P):
                            nc.tensor.matmul(
                                po[:, :cs], dw_t[:, fo, ds(dt_ * P, P)],
                                h_t[:, fo, ds(c0, cs)],
                                start=(fo == 0), stop=(fo == F // P - 1),
                            )
                        if ci % 2 == 0:
                            nc.scalar.activation(osb[:, ds(c0, cs)], po[:, :cs],
                                                 mybir.ActivationFunctionType.Copy)
                        else:
                            nc.vector.tensor_scalar_mul(osb[:, ds(c0, cs)], po[:, :cs], 1.0)
                        # Per-chunk DMA: subtile deps release each slice as
                        # soon as its copy lands.
                        nc.sync.dma_start(out_d[:, dt_, ds(c0, cs)], osb[:, ds(c0, cs)])
                else:
                    pos = [ps.tile([P, 512], F32, tag=f"{grp}{ci}", name=f"po{ci}")
                           for ci in range(nch)]
                    for fo in range(F // P):
                        for ci in range(nch):
                            nc.tensor.matmul(
                                pos[ci][:, :ck], dw_t[:, fo, ds(dt_ * P, P)],
                                h_t[:, fo, ds(ci * ck, ck)],
                                start=(fo == 0), stop=(fo == F // P - 1),
                            )
                    osb = stage.tile([P, cap], BF16, tag="osb", name="osb")
                    for ci in range(nch):
                        if ci % 2 == 0:
                            nc.scalar.activation(osb[:, ds(ci * ck, ck)], pos[ci][:, :ck],
                                                 mybir.ActivationFunctionType.Copy)
                        else:
                            nc.vector.tensor_scalar_mul(osb[:, ds(ci * ck, ck)], pos[ci][:, :ck], 1.0)
                    nc.sync.dma_start(out_d[:, dt_], osb[:])
    nc.finalize()
    return nc


def _route(gating_output):
    """Numpy softmax + top-2 + renormalize; returns (ids [T,K], w [T,K])."""
    g = gating_output.astype(np.float32)
    m = g.max(axis=-1, keepdims=True)
    e = np.exp(g - m)
    probs = e / e.sum(axis=-1, keepdims=True)
    ids = np.argsort(-probs, axis=-1, kind="stable")[:, :TOPK]
    w = np.take_along_axis(probs, ids, axis=-1)
    w = w / w.sum(axis=-1, keepdims=True)
    return ids, w


def kernel(x, gating_output, gate_w, up_w, down_w):
    x = np.asarray(x, dtype=np.float32)
    gating_output = np.asarray(gating_output, dtype=np.float32)
    gate_w = np.asarray(gate_w, dtype=np.float32)
    up_w = np.asarray(up_w, dtype=np.float32)
    down_w = np.asarray(down_w, dtype=np.float32)

    ids, w = _route(gating_output)

    idx_e = []
    w_e = []
    for e in range(E):
        sel = np.nonzero((ids == e).any(axis=-1))[0]
        kpos = (ids[sel] == e).argmax(axis=-1)
        idx_e.append(sel)
        w_e.append(w[sel, kpos])

    cap = max(len(i) for i in idx_e)
    nch = max(1, -(-cap // 512))  # token chunks (<=512 fp32 psum free dim)
    ck = -(-cap // nch)
    ck += ck & 1  # even chunk size
    cap_pad = nch * ck

    nc = _build(nch, ck)

    in_maps = []
    for e in range(E):
        idx = idx_e[e]
        cnt = len(idx)
        x_pad = np.zeros((cap_pad, D), dtype=np.float32)
        x_pad[:cnt] = x[idx]

        # x: [cap_pad, D] -> [128(dp), nch(ci), D/128(do), ck(c)] chunk-major
        x_dev = np.ascontiguousarray(
            x_pad.reshape(nch, ck, D // P, P).transpose(3, 0, 2, 1)).astype(BF)
        # gate/up: [F, D] -> T -> [D, F] -> [128(dp), 16(ft), 8(do), 128(fi)]
        gwT = gate_w[e].T  # [D, F]
        gw_dev = np.ascontiguousarray(
            gwT.reshape(D // P, P, F // P, P).transpose(1, 2, 0, 3)).astype(BF)
        uwT = up_w[e].T
        uw_dev = np.ascontiguousarray(
            uwT.reshape(D // P, P, F // P, P).transpose(1, 2, 0, 3)).astype(BF)
        # down: [D, F] -> T -> [F, D] -> [128(fp), 16(fo), D]
        dwT = down_w[e].T  # [F, D]
        dw_dev = np.ascontiguousarray(
            dwT.reshape(F // P, P, D).transpose(1, 0, 2)).astype(BF)

        in_maps.append({"x": x_dev, "gw": gw_dev, "uw": uw_dev, "dw": dw_dev})

    def _run():
        try:
            return run_bass_kernel_spmd(nc, in_maps, core_ids=list(range(N_CORES)))
        except Exception:
            # First execution of a fresh NEFF occasionally dies with
            # NRT_EXEC_UNIT_UNRECOVERABLE on this setup; the retry reuses
            # the cached executable and goes through.
            import time as _time

            _time.sleep(5)
            return run_bass_kernel_spmd(nc, in_maps, core_ids=list(range(N_CORES)))

    def _assemble(res):
        out = np.zeros((T, D), dtype=np.float32)
        for e in range(E):
            cnt = len(idx_e[e])
            # device out: [dp, do, c] -> [c, do*128+dp]
            o = res.results[e]["out"].astype(np.float32).transpose(2, 1, 0).reshape(cap_pad, D)
            out[idx_e[e]] += o[:cnt] * w_e[e][:, None]
        return out

    def _spot_check(out):
        # One token per (non-empty) expert, host-computed in f32. Catches the
        # rare corrupted execution (seen once: silently wrong rows on a fresh
        # NEFF) -- bf16 rounding keeps honest rows well under the threshold.
        worst = 0.0
        for e in range(E):
            if len(idx_e[e]) == 0:
                continue
            t = int(idx_e[e][0])
            acc = np.zeros(D, dtype=np.float32)
            for k in range(TOPK):
                ek = int(ids[t, k])
                g = gate_w[ek] @ x[t]
                u = up_w[ek] @ x[t]
                h = (g / (1.0 + np.exp(-g))) * u
                acc += w[t, k] * (down_w[ek] @ h)
            scale = np.abs(acc).max() + 1e-6
            worst = max(worst, np.abs(out[t] - acc).max() / scale)
        return worst

    res = _run()
    out = _assemble(res)
    if _spot_check(out) > 0.1:
        res = _run()
        out = _assemble(res)
    return out
